# revision 38
# baseline (speedup 1.0000x reference)
"""Trainium2 Bass kernel for nn_Experts (topk_masking) — v4.

Math (reference):
  R = concat(h,us,ue) @ W_r.T + b_r                       [1,1,512]
  x = concat(u, R.broadcast)                              [1,S,1536]
  h1 = (x @ W_nn.T + b_nn).reshape(S,512,16)
  h2 = (x @ W_no.T + b_no).reshape(S,512,16) * noise
  g  = top2-masked softmax over experts of (h1+h2)
  e  = (x @ W_E.T + b_E).reshape(S,512,16)
  out = (g*e).mean(-1)                                    [1,S,512]

Sharding: NE*DIM output-feature dim of the three projections sharded across
8 cores (64 dims x 16 experts per core); R-path folded to per-feature
constants on the host (fp64).

v4 changes vs the 54-matmul baseline (all validated against the fp64
reference on the real inputs, rel_l2 = 1.22e-2, gate 2e-2):
  * Zero bias matmuls and zero DVE m-add: the gating biases ride in a
    host-precomputed nzc = c_nn + c_no*noise stream (one DVE add); the
    noise product t = f32r(h2*noise) is accumulated into h1's PSUM bank by
    a K=128 identity matmul (213 ns vs 1127 ns on DVE), one block late so
    the PE never waits on DVE. e-bias via a K=2 head+residual matmul.
  * e-projection runs in fp16 (PE rate identical, DMA halved; e-errors
    never flip the top-2 selection - measured no rel_l2 change).
  * Engine budget per 128-token chunk: PE 52 matmul-equivalents (11.1 us),
    DVE 10.6 us, ACT 5.0 us; weights stream h0-first so the warmup is
    paced, and the last chunk's epilogue runs per-half to shrink the tail.
"""
import numpy as np

DIM = 512
NE = 16
S = 4096
KU = 2 * DIM        # u features = 1024
NCORES = 8
DL = DIM // NCORES  # 64 dims per core
FL = DL * NE        # 1024 features per core
MCH = S // 128      # 32 token chunks
NF = 512            # features per psum tile (half of FL)
NB = 2 * MCH        # 64 (chunk, half) blocks
NPAIR = MCH // 2

_MASK11 = np.uint32(0xFFFFF000)  # keep 11 explicit mantissa bits

TRACE = False
_CACHE = {}


def _trunc11(a):
    a = np.ascontiguousarray(a, dtype=np.float32)
    return (a.view(np.uint32) & _MASK11).view(np.float32)


def _round11(a):
    """Round-to-nearest (ties toward +ulp) at 11 explicit mantissa bits."""
    a = np.ascontiguousarray(a, dtype=np.float32)
    u = a.view(np.uint32).astype(np.uint64)
    lsb = (u >> np.uint64(12)) & np.uint64(1)
    u = u + np.uint64(0x7FF) + lsb
    u = u & np.uint64(0xFFFFF000)
    return u.astype(np.uint32).view(np.float32)


def _build():
    import concourse.mybir as mybir
    import concourse.tile as tile
    from concourse import bacc
    from contextlib import ExitStack

    F32 = mybir.dt.float32
    F32R = mybir.dt.float32r
    F16 = mybir.dt.float16
    BF16 = mybir.dt.bfloat16
    AX = mybir.AxisListType
    OP = mybir.AluOpType
    ACTF = mybir.ActivationFunctionType

    nc = bacc.Bacc("TRN2", target_bir_lowering=False, debug=False,
                   num_devices=NCORES)

    def dram(name, shape, dt, kind="ExternalInput"):
        return nc.dram_tensor(name, shape, dt, kind=kind)

    # per-core inputs (same names on every core; data differs per core)
    xrT = dram("xrT", [128, MCH, 8, 128], F16)    # x (all projections), [k,t]
    wnnT = dram("wnnT", [KU, FL], F16)
    wnoT = dram("wnoT", [KU, FL], F16)
    weT = dram("weT", [KU, FL], F16)
    nzP = dram("nzP", [128, NB * NF], F32)        # noise, (chunk,half) blocks
    nzcP = dram("nzcP", [128, NB * NF], F32R)      # c_nn + c_no*noise
    c3two = dram("c3two", [2, FL], F32R)          # e-bias head + residual
    identD = dram("identD", [128, 128], F32R)
    out_c = dram("out_c", [128, NB * 32], F32, kind="ExternalOutput")

    with tile.TileContext(nc) as tc, ExitStack() as ctx:
        wpool = ctx.enter_context(tc.tile_pool(name="w", bufs=1))

        # constants + chunk-0 stream first, then h0 weights, pair-0/1
        # streams, then h1 weights: the PE's h0-first block order consumes
        # weights at the pace they arrive instead of stalling on the full
        # 10 MiB preload
        with tc.high_priority(offset=400):
            ident_sb = wpool.tile([128, 128], F32R)
            nc.sync.dma_start(ident_sb[:], identD.ap())
            c3sb = wpool.tile([2, FL], F32R)
            nc.sync.dma_start(c3sb[:], c3two.ap())
        onesf = wpool.tile([2, 128], F32)
        nc.vector.memset(onesf[:], 1.0)
        ones2 = wpool.tile([2, 128], F32R)
        nc.vector.tensor_copy(ones2[:], onesf[:])

        wt = {}
        for h in range(2):
            for k in range(8):
                prio = (385 if k == 0 else 360) if h == 0 else 344
                with tc.high_priority(offset=prio):
                    for nm, dr, dt_ in (("nn", wnnT, F16), ("no", wnoT, F16),
                                        ("e", weT, F16)):
                        t = wpool.tile([128, NF], dt_, tag=f"w{nm}{h}{k}")
                        ksl = slice(k * 128, (k + 1) * 128)
                        fsl = slice(h * NF, (h + 1) * NF)
                        nc.sync.dma_start(t[:], dr.ap()[ksl, fsl])
                        wt[(nm, h, k)] = t

        spool = ctx.enter_context(tc.tile_pool(name="stream", bufs=4))
        epool = ctx.enter_context(tc.tile_pool(name="epi", bufs=2))
        mpsum = ctx.enter_context(tc.tile_pool(name="mps", bufs=2, space="PSUM"))

        xr_ts, nz_ts, nzc_ts = {}, {}, {}

        def prefetch_x(c, prio):
            with tc.high_priority(offset=prio):
                xr_t = spool.tile([128, 8, 128], F16, tag="xr")
                nc.sync.dma_start(xr_t[:], xrT.ap()[:, c])
            xr_ts[c] = xr_t

        def prefetch_nz(c, prio):
            with tc.high_priority(offset=prio):
                nz_t = spool.tile([128, 2 * NF], F32, tag="nz")
                nc.sync.dma_start(nz_t[:], nzP.ap()[:, 2 * c * NF:(2 * c + 2) * NF])
            nz_ts[c] = nz_t

        def prefetch_nzc(c, prio):
            with tc.high_priority(offset=prio):
                nzc_t = spool.tile([128, 2 * NF], F32R, tag="nzc")
                nc.sync.dma_start(nzc_t[:], nzcP.ap()[:, 2 * c * NF:(2 * c + 2) * NF])
            nzc_ts[c] = nzc_t

        def prefetch(c, prio):
            prefetch_x(c, prio)
            prefetch_nz(c, prio)
            prefetch_nzc(c, prio)

        t_c, msb_c, ecp_c, h1p_t = {}, {}, {}, {}
        e_done = set()

        def alloc_psum(c, h):
            h1p = mpsum.tile([128, NF], F32, tag="h1", bufs=3, name="h1p")
            h2p = mpsum.tile([128, NF], F32, tag="h2", bufs=2, name="h2p")
            ep = mpsum.tile([128, NF], F32, tag="ep", bufs=2, name="ep")
            h1p_t[(c, h)] = h1p
            return h1p, h2p, ep

        GE_SPLIT = {MCH - 1}   # final chunk: gating first, e-matmuls last

        def emit_h2_tail(c, h, h2p):
            """Evacuate h2 and launch the DVE noise product early."""
            fsl = slice(h * NF, (h + 1) * NF)
            h2sb = epool.tile([128, NF], F32, tag="h2sb", bufs=3)
            nc.scalar.copy(h2sb[:], h2p[:])
            if h == 0:
                t_c[c] = epool.tile([128, FL], F32R, tag="t", bufs=2, name="t_c")
            nc.vector.tensor_mul(t_c[c][:, fsl], h2sb[:], nz_ts[c][:, fsl])

        def emit_h1_tail(c, h, h1p):
            """nzc rides the PE as an identity matmul (its operand comes
            straight from DMA, so this never couples the PE to the DVE);
            msb = h1 + c_nn + c_no*noise after the evacuation. The last
            chunks keep h1p open: a deferred second identity matmul folds
            t in as well, so their epilogue skips the DVE m-add and the
            pipeline debt drains before the tail."""
            fsl = slice(h * NF, (h + 1) * NF)
            last = c not in TAILI
            nc.tensor.matmul(h1p[:], ident_sb[:], nzc_ts[c][:, fsl],
                             start=False, stop=last)
            if h == 0:
                msb_c[c] = epool.tile([128, FL], F32, tag="msb", bufs=2, name="msb")
            if last:
                nc.scalar.copy(msb_c[c][:, fsl], h1p[:])

        def emit_e_tail(c, h, ep):
            fsl = slice(h * NF, (h + 1) * NF)
            nc.tensor.matmul(ep[:], ones2[:], c3sb[:, fsl],
                             start=False, stop=True)
            if h == 0:
                ecp_c[c] = epool.tile([128, FL], BF16, tag="ecp", bufs=2, name="ecp")
            nc.scalar.copy(ecp_c[c][:, fsl], ep[:])

        def emit_g_part(c, h):
            """Gating matmuls, h2-projection first so t = h2*noise streams
            on the DVE while the h1 matmuls run."""
            h1p = mpsum.tile([128, NF], F32, tag="h1", bufs=3, name="h1p")
            h2p = mpsum.tile([128, NF], F32, tag="h2", bufs=2, name="h2p")
            h1p_t[(c, h)] = h1p
            for k in range(8):
                nc.tensor.matmul(h2p[:], xr_ts[c][:, k, :], wt[("no", h, k)][:],
                                 start=(k == 0), stop=(k == 7))
            emit_h2_tail(c, h, h2p)
            for k in range(8):
                nc.tensor.matmul(h1p[:], xr_ts[c][:, k, :], wt[("nn", h, k)][:],
                                 start=(k == 0), stop=False)
            emit_h1_tail(c, h, h1p)

        def emit_e_part(c, h):
            ep = mpsum.tile([128, NF], F32, tag="ep", bufs=2, name="ep")
            for k in range(8):
                nc.tensor.matmul(ep[:], xr_ts[c][:, k, :], wt[("e", h, k)][:],
                                 start=(k == 0), stop=False)
            emit_e_tail(c, h, ep)

        def emit_block(c, h):
            emit_g_part(c, h)
            emit_e_part(c, h)

        def emit_pair_kinterleaved(c0, h):
            """First-pair warmup: alternate k-matmuls across chunks c0, c0+1
            so the PE consumes each freshly-arrived weight triple twice."""
            ps = {c: alloc_psum(c, h) for c in (c0, c0 + 1)}
            for k in range(8):
                for c in (c0, c0 + 1):
                    h1p, h2p, ep = ps[c]
                    st = (k == 0)
                    nc.tensor.matmul(h1p[:], xr_ts[c][:, k, :],
                                     wt[("nn", h, k)][:], start=st, stop=False)
                    nc.tensor.matmul(h2p[:], xr_ts[c][:, k, :],
                                     wt[("no", h, k)][:], start=st, stop=(k == 7))
                    nc.tensor.matmul(ep[:], xr_ts[c][:, k, :],
                                     wt[("e", h, k)][:], start=st, stop=False)
            for c in (c0, c0 + 1):
                h1p, h2p, ep = ps[c]
                emit_h2_tail(c, h, h2p)
                emit_h1_tail(c, h, h1p)
                emit_e_tail(c, h, ep)

        epi_post = []

        def epilogue_pre(c, h0, n, lite=False):
            """Top-2 mask build over blocks (c,h0..h0+n-1): everything that
            does not need the ACT exponentials."""
            W = n * NF
            G = 32 * n
            col = slice(h0 * NF, h0 * NF + W)
            if lite:
                mf_ap = msb_c[c][:, col]    # t and nzc already folded on PE
            else:
                mfull = epool.tile([128, W], F32, tag="mf", bufs=2, name="mfull")
                nc.vector.tensor_add(mfull[:], msb_c[c][:, col], t_c[c][:, col])
                mf_ap = mfull[:]
            mg = mf_ap.rearrange("p (d e) -> p d e", e=NE)
            v1 = epool.tile([128, G], F32, tag="v1", bufs=2, name="v1")
            nc.vector.tensor_reduce(v1[:], mg, AX.X, op=OP.max)
            eq1 = epool.tile([128, W], BF16, tag="eq", bufs=1, name="eq1")
            nc.vector.tensor_tensor(eq1[:].rearrange("p (d e) -> p d e", e=NE),
                                    mg, v1[:].broadcast_to([128, G, NE]),
                                    OP.is_equal)
            m2 = epool.tile([128, W], F32, tag="m2", bufs=1, name="m2")
            nc.vector.scalar_tensor_tensor(m2[:], eq1[:], -1e30, mf_ap,
                                           OP.mult, OP.add)
            v2 = epool.tile([128, G], F32, tag="v2", bufs=2, name="v2")
            nc.vector.tensor_reduce(v2[:],
                                    m2[:].rearrange("p (d e) -> p d e", e=NE),
                                    AX.X, op=OP.max)
            mask = epool.tile([128, W], BF16, tag="mk", bufs=2, name="mask")
            nc.vector.tensor_tensor(mask[:].rearrange("p (d e) -> p d e", e=NE),
                                    mg, v2[:].broadcast_to([128, G, NE]),
                                    OP.is_ge)
            epi_post.append((c, h0, n, mf_ap, v1, v2, mask))

        def epilogue_post():
            """ACT exponentials + the weighted sum. Deferred one block so the
            in-order ACT queue never blocks the next block's evacuations."""
            c, h0, n, mf_ap, v1, v2, mask = epi_post.pop(0)
            W = n * NF
            G = 32 * n
            col = slice(h0 * NF, h0 * NF + W)
            t1 = epool.tile([128, W], BF16, tag="t1", bufs=1, name="t1")
            nc.vector.tensor_mul(t1[:], mask[:], ecp_c[c][:, col])
            q = epool.tile([128, W], BF16, tag="q", bufs=2, name="q")
            nc.scalar.activation(q[:], mf_ap, ACTF.Exp)
            ev = epool.tile([128, 2 * G], F32, tag="ev", bufs=2, name="ev")
            nc.scalar.activation(ev[:, :G], v1[:], ACTF.Exp)
            nc.scalar.activation(ev[:, G:], v2[:], ACTF.Exp)
            t2 = epool.tile([128, W], BF16, tag="t2", bufs=1, name="t2")
            nc.vector.tensor_mul(t2[:], t1[:], q[:])
            s_t = epool.tile([128, G], F32, tag="s", bufs=2, name="s_t")
            nc.vector.tensor_reduce(s_t[:],
                                    t2[:].rearrange("p (d e) -> p d e", e=NE),
                                    AX.X, op=OP.add)
            z_t = epool.tile([128, G], F32, tag="z", bufs=1, name="z_t")
            nc.vector.tensor_add(z_t[:], ev[:, :G], ev[:, G:])
            r_t = epool.tile([128, G], F32, tag="r", bufs=1, name="r_t")
            nc.vector.reciprocal(r_t[:], z_t[:])
            oo = epool.tile([128, G], F32, tag="oo", bufs=3, name="oo")
            nc.vector.scalar_tensor_tensor(oo[:], s_t[:], 1.0 / NE, r_t[:],
                                           OP.mult, OP.mult)
            b0 = 2 * c + h0
            nc.sync.dma_start(out_c.ap()[:, b0 * 32:(b0 + n) * 32], oo[:])

        def release(c):
            xr_ts.pop(c), nz_ts.pop(c), nzc_ts.pop(c)

        # Warmup DMA order (priority-controlled): consts/x(0,1), h0 weights
        # (@360), noise(0,1), h1 weights (@344), x(2,3), noise(2,3), nzc.
        prefetch_x(0, 390)
        prefetch_x(1, 384)
        prefetch_nzc(0, 353)
        prefetch_nzc(1, 352)
        prefetch_x(2, 351)
        prefetch_x(3, 350)
        prefetch_nz(0, 342)
        prefetch_nz(1, 341)
        prefetch_nzc(2, 340)
        prefetch_nzc(3, 339)
        prefetch_nz(2, 338)
        prefetch_nz(3, 337)

        seq = [("pair0", 0, 0), ("pair0", 0, 1)]
        for i in range(1, NPAIR - 2):
            seq += [("blk", 2 * i, 0), ("blk", 2 * i + 1, 0),
                    ("blk", 2 * i, 1), ("blk", 2 * i + 1, 1)]
        seq += [("blk", MCH - 4, 0), ("blk", MCH - 4, 1)]  # noqa - pairs end at 27
        seq += [("blk", MCH - 3, 0), ("blk", MCH - 3, 1),
                ("blk", MCH - 2, 0), ("blk", MCH - 2, 1),
                ("g", MCH - 1, 0), ("g", MCH - 1, 1),
                ("e", MCH - 1, 0), ("e", MCH - 1, 1)]

        TAILI = set(range(MCH - 6, MCH))
        pend2 = []

        def post_ready():
            c, h0, n = epi_post[0][0], epi_post[0][1], epi_post[0][2]
            return all((c, h0 + i) in e_done for i in range(n))

        def tiadd_pop():
            cc, hh = pend2.pop(0)
            fsl = slice(hh * NF, (hh + 1) * NF)
            nc.tensor.matmul(h1p_t[(cc, hh)][:], ident_sb[:],
                             t_c[cc][:, fsl], start=False, stop=True)
            nc.scalar.copy(msb_c[cc][:, fsl], h1p_t[(cc, hh)][:])
            if cc >= MCH - 2:
                epilogue_pre(cc, hh, 1, lite=True)
            elif hh == 1:
                epilogue_pre(cc, 0, 2, lite=True)
                if cc < MCH - 3:   # the split chunks still need xr for e
                    release(cc)

        for kind, c, h in seq:
            if kind == "pair0":
                emit_pair_kinterleaved(0, h)
                if h == 1:
                    epilogue_pre(0, 0, 2)
                    release(0)
                    epilogue_pre(1, 0, 2)
                    release(1)
            else:
                if kind == "blk":
                    if h == 0 and c % 2 == 0 and c + 3 < MCH:
                        prefetch(c + 2, 300)
                        prefetch(c + 3, 300)
                    emit_block(c, h)
                elif kind == "g":
                    emit_g_part(c, h)
                else:
                    emit_e_part(c, h)
                while epi_post and post_ready():
                    epilogue_post()
                if pend2:
                    tiadd_pop()
                if kind != "e" and c in TAILI:
                    pend2.append((c, h))
                elif kind == "blk" and h == 1:
                    epilogue_pre(c, 0, 2)
                    release(c)
        while pend2:
            tiadd_pop()
        while epi_post:
            epilogue_post()

    nc.compile()
    return nc


def _get_program():
    if "nc" not in _CACHE:
        _CACHE["nc"] = _build()
    return _CACHE["nc"]


def kernel(h, us, ue, u, noise, W_nn, b_nn, W_no, b_no, W_E, b_E, W_r, b_r):
    from concourse.bass_utils import run_bass_kernel_spmd

    f32 = np.float32
    u2 = np.ascontiguousarray(np.asarray(u, dtype=f32).reshape(S, KU))
    # [p, chunk, kc, t]: feature kc*128+p, token chunk*128+t
    xrT = np.ascontiguousarray(
        u2.astype(np.float16).T.reshape(8, 128, MCH, 128).transpose(1, 2, 0, 3))

    # token-independent R path on host (fp64): R then c = R @ W[:,KU:].T + b
    hx = np.concatenate([np.asarray(h, dtype=np.float64).ravel(),
                         np.asarray(us, dtype=np.float64).ravel(),
                         np.asarray(ue, dtype=np.float64).ravel()])
    R = hx @ np.asarray(W_r, dtype=np.float64).T + np.asarray(b_r, np.float64)

    W_nn = np.asarray(W_nn, dtype=f32)
    W_no = np.asarray(W_no, dtype=f32)
    W_E = np.asarray(W_E, dtype=f32)
    c_nn = (R @ W_nn[:, KU:].T.astype(np.float64)
            + np.asarray(b_nn, np.float64)).astype(f32)
    c_no = (R @ W_no[:, KU:].T.astype(np.float64)
            + np.asarray(b_no, np.float64)).astype(f32)
    c_E = (R @ W_E[:, KU:].T.astype(np.float64)
           + np.asarray(b_E, np.float64)).astype(f32)
    noise4 = np.asarray(noise, dtype=f32).reshape(S, DIM, NE)
    identD = np.ascontiguousarray(np.eye(128, dtype=f32))

    def pack(a):
        # [S, FL] -> [128, NB*NF] with (chunk, half) blocks along free dim
        return np.ascontiguousarray(
            a.reshape(MCH, 128, 2, NF).transpose(1, 0, 2, 3).reshape(128, -1))

    in_maps = []
    for c in range(NCORES):
        fsl = slice(c * FL, (c + 1) * FL)
        dsl = slice(c * DL, (c + 1) * DL)
        nzsl = np.ascontiguousarray(noise4[:, dsl, :].reshape(S, FL))
        ce = c_E[fsl]
        c3h = _trunc11(ce)
        im = {
            "xrT": xrT,
            "wnnT": np.ascontiguousarray(W_nn[fsl, :KU].astype(np.float16).T),
            "wnoT": np.ascontiguousarray(W_no[fsl, :KU].astype(np.float16).T),
            "weT": np.ascontiguousarray(W_E[fsl, :KU].astype(np.float16).T),
            "nzP": pack(nzsl),
            "nzcP": pack((c_nn[fsl][None] + c_no[fsl][None] * nzsl).astype(f32)),
            "c3two": np.ascontiguousarray(
                np.stack([c3h, _round11((ce - c3h).astype(f32))])),
            "identD": identD,
        }
        in_maps.append(im)

    nc = _get_program()
    res = run_bass_kernel_spmd(nc, in_maps, core_ids=list(range(NCORES)),
                               trace=TRACE)
    _CACHE["last_results"] = res
    out = np.empty((1, S, DIM), dtype=f32)
    for c in range(NCORES):
        r = res.results[c]["out_c"].reshape(128, MCH, 2, 32)
        out[0, :, c * DL:(c + 1) * DL] = (
            r.transpose(1, 0, 2, 3).reshape(S, DL))
    return out
